# revision 13
# baseline (speedup 1.0000x reference)
"""Multi-head attention (B=4, S=2048, D=1024, H=16, dk=dv=64) on 8 TRN2 cores.

Sharding: core c = 2*b + hg handles batch b = c//2 and heads
[hg*8, hg*8+8). Each core computes a partial output
(its 8 heads' contribution through Wo); the host adds the two partials
per batch.

Per-core device pipeline (matmul inputs bf16, PSUM accumulation fp32):
  1. Prelude: khT projection (all blocks), vh projection, qhT block 0.
     khT/qhT pair layout: partitions 0-63 = h0's dk, 64-127 = h1's dk.
     vh stored per key-chunk as [128 tok, 8*65] bf16 (64 dv cols + a
     mask/ones col per head, masked keys zeroed).
  2. scores^T per head pair via 64x128 PE row tiling: h0 on tile
     (0,0) (SBUF partitions 0-63), h1 on tile (64,0) (partitions
     64-127); the two matmuls run concurrently on the PE array.
     Output [128 keys, 512 q] fp32 in PSUM, two key-chunks per tile.
  3. exp on ScalarE PSUM->SBUF bf16. The emission is software-
     pipelined so ScalarE never waits: scores(g+1) [or the next
     pair's scores(0)] are emitted between exp(g) and mix(g).
  4. mix^T + softmax sums in one matmul: lhsT = vh_aug [128 keys, 65]
     (col 64 = mask), rhs = exp chunk half [128, 512]; accumulate all
     16 key chunks into one PSUM bank per head (rows 0-64).
  5. normalize: madd = PSUM rows 0-64 -> SBUF fp32 (DVE); sums row 64
     broadcast across partitions with a float32r PE matmul (lhsT =
     e65, K=65, only row 64 ones); reciprocal + multiply (DVE, out
     bf16). h1's tile is DMA-shifted to partitions 64-127 so each
     pair's normalized mix^T is one [128, 512] tile (e on partitions).
  6. out += mixT_norm.T @ Wo: dense K=128 bf16 matmuls accumulating
     over the 4 pairs; DVE evac fp32 -> DMA to HBM. Wo work for block
     qb and the q projection for block qb+1 are deferred thunks run
     inside later pairs' g-loops to fill PE slack under ScalarE.
"""

import numpy as np

B, S, D = 4, 2048, 1024
H, DK, DV = 16, 64, 64
HC = 8          # heads per core
NP = HC // 2    # head pairs per core
NCORES = 8
NC_CHUNKS = D // 128    # 8 contraction chunks over D
NKC = S // 128          # 16 key chunks
NQB = S // 512          # 4 query blocks
NG = NKC // 2           # score/exp groups per pair (2 key chunks each)
VW = HC * 65            # vh storage: 65 cols per head (dv | mask)

_COMPILED = {}

_E65 = np.zeros((128, DV + 1), np.float32)
_E65[64, :] = 1.0


def _build_nc():
    import concourse.tile as tile
    from concourse import bacc, mybir
    from contextlib import ExitStack

    F32 = mybir.dt.float32
    F32R = mybir.dt.float32r
    BF16 = mybir.dt.bfloat16
    EXP = mybir.ActivationFunctionType.Exp

    nc = bacc.Bacc("TRN2", target_bir_lowering=False, debug=False,
                   num_devices=NCORES)

    qT = nc.dram_tensor("qT", [D, S], BF16, kind="ExternalInput").ap()
    kT = nc.dram_tensor("kT", [D, S], BF16, kind="ExternalInput").ap()
    vT = nc.dram_tensor("vT", [D, S], BF16, kind="ExternalInput").ap()
    wq = nc.dram_tensor("wq", [D, HC * DK], BF16, kind="ExternalInput").ap()
    wk = nc.dram_tensor("wk", [D, HC * DK], BF16, kind="ExternalInput").ap()
    wv = nc.dram_tensor("wv", [D, HC * DV], BF16, kind="ExternalInput").ap()
    wo = nc.dram_tensor("wo", [HC * DV, D], BF16, kind="ExternalInput").ap()
    maskr = nc.dram_tensor("maskr", [128, NKC], F32, kind="ExternalInput").ap()
    e65r = nc.dram_tensor("e65", [128, DV + 1], F32R,
                          kind="ExternalInput").ap()
    out = nc.dram_tensor("out", [S, D], F32, kind="ExternalOutput").ap()

    with tile.TileContext(nc) as tc:
        with ExitStack() as ctx:
            const_pool = ctx.enter_context(tc.tile_pool(name="const", bufs=1))
            w_pool = ctx.enter_context(tc.tile_pool(name="weights", bufs=1))
            act_pool = ctx.enter_context(tc.tile_pool(name="acts", bufs=1))
            st_pool = ctx.enter_context(
                tc.tile_pool(name="stage", bufs=2 * NC_CHUNKS))

            # PSUM pools: 4 + 2 + 2 = 8 banks exactly. Scores use one
            # persistent 4-bank tile, ping-ponged by halves (2 banks per
            # score group) so scores(g+1) overlaps exp(g).
            sc_pool = ctx.enter_context(
                tc.tile_pool(name="scpsum", bufs=1, space="PSUM"))
            mx_pool = ctx.enter_context(
                tc.tile_pool(name="mxpsum", bufs=2, space="PSUM"))
            util_pool = ctx.enter_context(
                tc.tile_pool(name="utpsum", bufs=2, space="PSUM"))

            exp_pool = ctx.enter_context(tc.tile_pool(name="exp", bufs=6))
            norm_pool = ctx.enter_context(tc.tile_pool(name="norm", bufs=8))
            tmp_pool = ctx.enter_context(tc.tile_pool(name="tmp", bufs=10))
            out_pool = ctx.enter_context(tc.tile_pool(name="outsb", bufs=4))

            mask_sb = const_pool.tile([128, NKC], F32)
            nc.sync.dma_start(mask_sb[:], maskr[:])
            ones_sb = const_pool.tile([128, 64], BF16)
            nc.vector.memset(ones_sb[:], 1.0)
            # bcast helper: only row 64 ones -> out rows = sums row replicated
            e65_sb = const_pool.tile([128, DV + 1], F32R)
            nc.sync.dma_start(e65_sb[:], e65r[:])

            wq_sb = w_pool.tile([128, NC_CHUNKS * 512], BF16, tag="wq")
            wk_sb = w_pool.tile([128, NC_CHUNKS * 512], BF16, tag="wk")
            wv_sb = w_pool.tile([128, NC_CHUNKS * 512], BF16, tag="wv")
            wo_sb = w_pool.tile([128, NP * 1024], BF16, tag="wo")
            for c in range(NC_CHUNKS):
                nc.sync.dma_start(wk_sb[:, c * 512:(c + 1) * 512],
                                  wk[c * 128:(c + 1) * 128, :])
                nc.sync.dma_start(wv_sb[:, c * 512:(c + 1) * 512],
                                  wv[c * 128:(c + 1) * 128, :])
                nc.sync.dma_start(wq_sb[:, c * 512:(c + 1) * 512],
                                  wq[c * 128:(c + 1) * 128, :])
            for p in range(NP):
                nc.sync.dma_start(wo_sb[:, p * 1024:(p + 1) * 1024],
                                  wo[p * 128:(p + 1) * 128, :])

            # persistent activations
            qhT = [act_pool.tile([128, S], BF16, tag=f"qhT{p}", name=f"qhT{p}")
                   for p in range(NP)]
            khT = [act_pool.tile([128, S], BF16, tag=f"khT{p}",
                                 name=f"khT{p}") for p in range(NP)]
            vhs = [act_pool.tile([128, VW], BF16, tag=f"vh{t}", name=f"vh{t}")
                   for t in range(NKC)]
            vt_sb = [act_pool.tile([128, S], BF16, tag=f"vt{c}", name=f"vt{c}")
                     for c in range(NC_CHUNKS)]

            def proj_block(src, wsb, dst, qb):
                """Project one 512-token block of q or k into dst[p]."""
                stg = []
                for c in range(NC_CHUNKS):
                    t = st_pool.tile([128, 512], BF16, tag="stage",
                                     name=f"stg{c}")
                    nc.sync.dma_start(
                        t[:], src[c * 128:(c + 1) * 128,
                                  qb * 512:(qb + 1) * 512])
                    stg.append(t)
                for p in range(NP):
                    proj_p(stg, wsb, dst, qb, p)

            def proj_p(stg, wsb, dst, qb, p):
                ps = util_pool.tile([128, 512], F32, tag="ut")
                for c in range(NC_CHUNKS):
                    nc.tensor.matmul(
                        ps[:],
                        lhsT=wsb[:, c * 512 + p * 128:
                                 c * 512 + (p + 1) * 128],
                        rhs=stg[c][:],
                        start=(c == 0), stop=(c == NC_CHUNKS - 1))
                qsl = slice(qb * 512, (qb + 1) * 512)
                nc.vector.tensor_copy(dst[p][:, qsl], ps[:])

            def vproj_t(t):
                """Project key-chunk t of v into vhs[t] (mask folded)."""
                ps = util_pool.tile([128, 512], F32, tag="ut")
                for c in range(NC_CHUNKS):
                    nc.tensor.matmul(
                        ps[:],
                        lhsT=vt_sb[c][:, t * 128:(t + 1) * 128],
                        rhs=wv_sb[:, c * 512:(c + 1) * 512],
                        start=(c == 0), stop=(c == NC_CHUNKS - 1))
                dst_dv = vhs[t][:, 0:VW].rearrange(
                    "p (h x) -> p h x", x=65)[:, :, 0:DV]
                src_dv = ps[:].rearrange("p (h x) -> p h x", x=DV)
                nc.vector.tensor_scalar_mul(dst_dv, src_dv,
                                            mask_sb[:, t:t + 1])
                dst_m = vhs[t][:, 0:VW].rearrange(
                    "p (h x) -> p h x", x=65)[:, :, DV:DV + 1]
                src_m = ones_sb[:, 0:HC].rearrange("p (h x) -> p h x", x=1)
                nc.vector.tensor_scalar_mul(dst_m, src_m,
                                            mask_sb[:, t:t + 1])

            # ---- prelude: k (all), v (all), q block 0 ----
            for kb in range(NQB):
                proj_block(kT, wk_sb, khT, kb)
            for c in range(NC_CHUNKS):
                for tb in range(NQB):
                    nc.sync.dma_start(
                        vt_sb[c][:, tb * 512:(tb + 1) * 512],
                        vT[c * 128:(c + 1) * 128, tb * 512:(tb + 1) * 512])
            for t in range(NKC):
                vproj_t(t)
            proj_block(qT, wq_sb, qhT, 0)

            # ---- attention + output projection ----
            sc_ps = sc_pool.tile([128, 2048], F32, tag="scps")
            sc_state = {"flip": 0}

            def emit_scores(qb, p, kc, scs):
                """Row-tiled scores for one key chunk, both heads.

                h0's [128 keys, 512 q] goes to the flip half's first
                bank, h1's to its second; the two matmuls run
                concurrently on PE row tiles (0,0) and (64,0)."""
                qful = qhT[p][:, qb * 512:(qb + 1) * 512]
                f = sc_state["flip"]
                sc_state["flip"] = 1 - f
                sc = sc_ps[:, f * 1024:f * 1024 + 1024]
                ksl = slice(kc * 128, (kc + 1) * 128)
                nc.tensor.matmul(
                    sc[:, 0:512],
                    lhsT=khT[p][0:64, ksl], rhs=qful[0:64, :],
                    start=True, stop=True)
                nc.tensor.matmul(
                    sc[:, 512:1024],
                    lhsT=khT[p][64:128, ksl], rhs=qful[64:128, :],
                    start=True, stop=True)
                scs.append(sc)

            def emit_wo_tt(qb, normT, tt):
                """One eighth of the Wo projection for query block qb."""
                tt4, dh = tt // 2, tt % 2
                wps = util_pool.tile([128, 512], F32, tag="ut")
                for p in range(NP):
                    nc.tensor.matmul(
                        wps[:],
                        lhsT=normT[p][:, tt4 * 128:(tt4 + 1) * 128],
                        rhs=wo_sb[:, p * 1024 + dh * 512:
                                  p * 1024 + (dh + 1) * 512],
                        start=(p == 0), stop=(p == NP - 1))
                osb = out_pool.tile([128, 512], F32, tag="osb")
                nc.vector.tensor_copy(osb[:], wps[:])
                nc.sync.dma_start(
                    out[qb * 512 + tt4 * 128:qb * 512 + (tt4 + 1) * 128,
                        dh * 512:(dh + 1) * 512], osb[:])

            q_stg = {}          # staging tiles for deferred q projections
            prev_norm = None    # (qb-1, normT list) for deferred Wo
            cur_scs = []
            emit_scores(0, 0, 0, cur_scs)

            for qb in range(NQB):
                # deferred PE work to run inside this qb's g-loops
                thunks = []
                if qb + 1 < NQB:
                    stg = []
                    for c in range(NC_CHUNKS):
                        t = st_pool.tile([128, 512], BF16, tag="stage",
                                         name=f"stg{c}")
                        nc.sync.dma_start(
                            t[:], qT[c * 128:(c + 1) * 128,
                                     (qb + 1) * 512:(qb + 2) * 512])
                        stg.append(t)
                    q_stg[qb + 1] = stg
                    for p in range(NP):
                        thunks.append(
                            lambda p=p, qb=qb: proj_p(
                                q_stg[qb + 1], wq_sb, qhT, qb + 1, p))
                if prev_norm is not None:
                    pq, pnorm = prev_norm
                    for tt in range(8):
                        thunks.append(
                            lambda pq=pq, pnorm=pnorm, tt=tt: emit_wo_tt(
                                pq, pnorm, tt))

                normT = []
                for p in range(NP):
                    h0, h1 = 2 * p, 2 * p + 1
                    l0 = slice(h0 * 65, h0 * 65 + 65)
                    l1 = slice(h1 * 65, h1 * 65 + 65)
                    scs = cur_scs
                    mixP = mx_pool.tile([128, 512], F32, tag="mx")
                    mixR = mx_pool.tile([128, 512], F32, tag="mx")
                    for kc in range(NKC):
                        ex = exp_pool.tile([128, 1024], BF16, tag="exp")
                        nc.scalar.activation(ex[:], scs[kc][:], EXP)
                        # keep ScalarE fed: emit the next scores chunk now
                        if kc + 1 < NKC:
                            emit_scores(qb, p, kc + 1, scs)
                        elif (qb, p) != (NQB - 1, NP - 1):
                            nqb, np_ = (qb, p + 1) if p + 1 < NP else (qb + 1, 0)
                            cur_scs = []
                            emit_scores(nqb, np_, 0, cur_scs)
                        if kc % 2 == 0 and 2 <= kc <= 12 and thunks:
                            thunks.pop(0)()
                        va = vhs[kc]
                        st = (kc == 0)
                        sp = (kc == NKC - 1)
                        nc.tensor.matmul(
                            mixP[0:65, :],
                            lhsT=va[:, l0], rhs=ex[:, 0:512],
                            start=st, stop=sp)
                        nc.tensor.matmul(
                            mixR[0:65, :],
                            lhsT=va[:, l1], rhs=ex[:, 512:1024],
                            start=st, stop=sp)
                    # normalize (sums broadcast via f32r PE matmul, K=65)
                    nt = norm_pool.tile([128, 512], BF16, tag="norm")
                    normT.append(nt)
                    madd0 = tmp_pool.tile([128, 512], F32R, tag="madd")
                    madd1 = tmp_pool.tile([128, 512], F32R, tag="madd")
                    nc.vector.tensor_copy(madd0[0:65, :], mixP[0:65, :])
                    nc.vector.tensor_copy(madd1[0:65, :], mixR[0:65, :])
                    bc0 = util_pool.tile([128, 512], F32, tag="ut")
                    bc1 = util_pool.tile([128, 512], F32, tag="ut")
                    nc.tensor.matmul(
                        bc0[0:DV + 1, :],
                        lhsT=e65_sb[0:DV + 1, 0:DV + 1],
                        rhs=madd0[0:DV + 1, :],
                        start=True, stop=True)
                    nc.tensor.matmul(
                        bc1[0:DV + 1, :],
                        lhsT=e65_sb[0:DV + 1, 0:DV + 1],
                        rhs=madd1[0:DV + 1, :],
                        start=True, stop=True)
                    rec0 = tmp_pool.tile([64, 512], F32, tag="rec")
                    rec1 = tmp_pool.tile([64, 512], F32, tag="rec")
                    nc.vector.reciprocal_approx_fast(rec0[:], bc0[0:64, :])
                    nc.vector.reciprocal_approx_fast(rec1[:], bc1[0:64, :])
                    nc.vector.tensor_mul(nt[0:64, :], madd0[0:64, :],
                                         rec0[:])
                    sh1 = tmp_pool.tile([64, 512], BF16, tag="sh1")
                    nc.vector.tensor_mul(sh1[:], madd1[0:64, :],
                                         rec1[:])
                    nc.sync.dma_start(nt[64:128, :], sh1[:])

                # any thunks not consumed inside the g-loops
                for th in thunks:
                    th()
                thunks = []
                prev_norm = (qb, normT)

            # last query block's Wo
            pq, pnorm = prev_norm
            for tt in range(8):
                emit_wo_tt(pq, pnorm, tt)

    nc.compile()
    return nc


def _get_nc():
    if "nc" not in _COMPILED:
        _COMPILED["nc"] = _build_nc()
    return _COMPILED["nc"]


def _shard_inputs(q, k, v, mask, Wq, Wk, Wv, Wo):
    """Build the per-core input maps (host-side layout prep)."""
    import ml_dtypes

    bf16 = ml_dtypes.bfloat16
    in_maps = []
    maskf = np.asarray(mask).astype(np.float32)
    q = np.asarray(q, np.float32)
    k = np.asarray(k, np.float32)
    v = np.asarray(v, np.float32)
    Wq = np.asarray(Wq, np.float32)
    Wk = np.asarray(Wk, np.float32)
    Wv = np.asarray(Wv, np.float32)
    Wo = np.asarray(Wo, np.float32)
    scale = np.float32(1.0 / np.sqrt(DK))
    for c in range(NCORES):
        b, hg = c // 2, c % 2
        hs = hg * HC
        m = {
            "qT": np.ascontiguousarray(q[b].T).astype(bf16),
            "kT": np.ascontiguousarray(k[b].T).astype(bf16),
            "vT": np.ascontiguousarray(v[b].T).astype(bf16),
            # head-major col blocks; fold 1/sqrt(dk) into Wq
            "wq": np.ascontiguousarray(
                Wq[hs:hs + HC].transpose(1, 0, 2).reshape(D, HC * DK) * scale
            ).astype(bf16),
            "wk": np.ascontiguousarray(
                Wk[hs:hs + HC].transpose(1, 0, 2).reshape(D, HC * DK)
            ).astype(bf16),
            "wv": np.ascontiguousarray(
                Wv[hs:hs + HC].transpose(1, 0, 2).reshape(D, HC * DV)
            ).astype(bf16),
            "wo": np.ascontiguousarray(Wo[hs * DV:(hs + HC) * DV]).astype(bf16),
            "maskr": np.ascontiguousarray(
                maskf[b].reshape(NKC, 128).T).astype(np.float32),
            "e65": _E65,
        }
        in_maps.append(m)
    return in_maps


def kernel(q, k, v, mask, Wq, Wk, Wv, Wo, _trace=False):
    from concourse.bass_utils import run_bass_kernel_spmd

    nc = _get_nc()
    in_maps = _shard_inputs(q, k, v, mask, Wq, Wk, Wv, Wo)
    res = run_bass_kernel_spmd(nc, in_maps, list(range(NCORES)),
                               trace=_trace)
    out = np.zeros((B, S, D), np.float32)
    for c in range(NCORES):
        out[c // 2] += res.results[c]["out"]
    if _trace:
        _COMPILED["last_result"] = res
    return out
